# revision 9
# baseline (speedup 1.0000x reference)
"""Trainium2 Bass kernel for nn_Event: per-example resonance synthesis.

Math (see derivation in comments below):
  res   = balance @ bank                                   (B=128, N=32768)
  res_sig = res * env          -- the window/decay overlap-add collapses to an
                                  elementwise envelope since decay is constant
                                  per example and hamming windows overlap 2:1
  impulse = bandpass(imp_sig, filt)   512-tap grouped correlation
  out = (res_sig (*) impulse) + pad(impulse), all scaled by amp
        where (*) is a 4096-tap grouped correlation with pad 2048.

Device strategy (8 cores, 16 examples each, pure data parallel):
  - correlations are computed on the TensorEngine as 33 (resp. 5)
    accumulating 128x128 Toeplitz-block matmuls per example.  The Toeplitz
    weight tiles are slices of a "shifted replication" matrix
    h_shift[j, c] = h_pad[c + j] read from DRAM with an overlapping
    (element-stride) access pattern, one DMA per example.
  - outputs are produced in a within-chunk-reversed layout [dt', n] with
    t = 128*n + 127 - dt' (this keeps every DMA/AP stride positive); the
    host undoes the permutation when gathering.
  - amp is folded into imp_sig on the host, so conv and impulse come out
    pre-scaled; the host adds the impulse head into the final signal.
"""

import numpy as np
import ml_dtypes
from numpy.lib.stride_tricks import sliding_window_view

import concourse.bass as bass
import concourse.tile as tile
import concourse.mybir as mybir
import bass_rust
from concourse import bass_utils
from concourse.vector_clock import ScopedClock

V = bass_rust.VecI64Pair
BF16 = mybir.dt.bfloat16
F32 = mybir.dt.float32

B = 128
N = 32768
NF = 128
IMP = 4096
FILT = 512
WIN = 512
HOP = 256
NCORES = 8
EX = B // NCORES          # 16 examples per core
NCH = N // 128            # 256 signal chunks
ICH = IMP // 128          # 32 impulse chunks
QBIG = 33                 # Toeplitz blocks for the 4096-tap conv
QBP = 5                   # Toeplitz blocks for the 512-tap bandpass
WBIG = QBIG * 128         # 4224
WBP = QBP * 128           # 640
HBUF = 128 + IMP + 384    # padded h buffer per example (4608)
GBUF = 128 + FILT + 256   # padded g buffer per example (896)


class PatchedTileContext(tile.TileContext):
    """TileContext adapted to a walrus build that rejects more than one sync
    wait per instruction: extra waits are peeled onto standalone single-wait
    nops committed just before their instruction; same for the tail drain."""

    def _commit_instruction(self, inst, lazy_reg_writes: bool = True):
        si = getattr(inst, "sync_info", None)
        if (
            si is not None
            and si.on_wait
            and len(si.on_wait) > 1
            and inst.engine != mybir.EngineType.Unassigned
        ):
            waits = list(si.on_wait)
            inst.sync_info = mybir.SyncInfo(
                on_wait=waits[-1:], on_update=list(si.on_update or [])
            )
            for w in waits[:-1]:
                n = mybir.InstNoOp(
                    name=self.nc.get_next_instruction_name(),
                    sync_info=mybir.SyncInfo(on_wait=[w], on_update=[]),
                    bass_nofuse=True,
                    engine=inst.engine,
                    text_hint="split_wait",
                )
                super()._commit_instruction(n, lazy_reg_writes=False)
        return super()._commit_instruction(inst, lazy_reg_writes)

    def _drain_and_barrier(self, tick_clock, wait_clock):
        nc = self.nc
        drain_inst = nc.sync.drain()
        wait_clock.add_sem_waits(
            drain_inst.ins, ScopedClock({None: tick_clock.global_clock})
        )
        si = drain_inst.ins.sync_info
        waits = list(si.on_wait) if si is not None and si.on_wait else []
        if len(waits) > 1:
            si.on_wait = []
            for w in waits:
                n = nc.sync.nop(nofuse=True, hint="split_drain_wait")
                n.ins.sync_info = mybir.SyncInfo(on_wait=[w], on_update=[])
        nc.all_engine_barrier()
        assert self.sems is not None
        popped = nc._tile_sem_poison_stack.pop()
        assert popped is self._sem_poison
        nc.clear_and_free_semaphores(list(self.sems.allocated().values()))
        nc.all_engine_barrier()


def _build_program():
    nc = bass.Bass("TRN2", target_bir_lowering=False, debug=False)

    balT = nc.dram_tensor("balT", [NF, EX], BF16, kind="ExternalInput")
    bank = nc.dram_tensor("bank", [NF, N], BF16, kind="ExternalInput")
    envv = nc.dram_tensor("envv", [EX, 128, NCH], BF16, kind="ExternalInput")
    x2in = nc.dram_tensor("x2in", [EX, 128, ICH + 4], BF16, kind="ExternalInput")
    gsin = nc.dram_tensor("gsin", [EX, 128, WBP], BF16, kind="ExternalInput")
    outp = nc.dram_tensor("outp", [EX, 128, NCH], F32, kind="ExternalOutput")
    impo = nc.dram_tensor("impo", [EX, IMP], BF16, kind="ExternalOutput")

    with PatchedTileContext(nc) as tc:
        with (
            tc.tile_pool(name="const", bufs=1) as constp,
            tc.tile_pool(name="dram", bufs=1, space="DRAM") as dramp,
            tc.tile_pool(name="bankp", bufs=6) as bankp,
            tc.tile_pool(name="work", bufs=4) as work,
            tc.tile_pool(name="hsp", bufs=4) as hsp,
            tc.tile_pool(name="outs", bufs=3) as outs,
            tc.tile_pool(name="psr", bufs=3, space="PSUM") as psr,
            tc.tile_pool(name="ps1", bufs=3, space="PSUM") as ps1,
            tc.tile_pool(name="ps2", bufs=2, space="PSUM") as ps2,
        ):
            h_buf = dramp.tile([EX, HBUF], BF16)

            balT_sb = constp.tile([NF, EX], BF16)
            nc.sync.dma_start(out=balT_sb[:], in_=balT.ap())

            zero_sb = constp.tile([EX, 384], BF16)
            nc.vector.memset(zero_sb[:], 0.0)
            nc.gpsimd.dma_start(out=h_buf[:, 0:128], in_=zero_sb[:, 0:128])
            nc.gpsimd.dma_start(out=h_buf[:, 128 + IMP:HBUF], in_=zero_sb[:])

            # ---- Phase A: resT[t, b] = (balance @ bank).T, kept on-chip ----
            # resT_sb column 16*m + b holds res[b, 128*m + j] on partition j.
            resT_sb = constp.tile([128, EX * NCH], BF16)
            for gl in range(16):
                bank_sb = bankp.tile([NF, 2048], BF16, tag="bank_sb")
                ldeng = nc.scalar if gl % 2 == 0 else nc.sync
                ldeng.dma_start(out=bank_sb[:], in_=bank.ap()[:, 2048 * gl:2048 * (gl + 1)])
                for q4 in range(4):
                    psA = psr.tile([128, 4 * EX], F32, tag="psA")
                    for mq in range(4):
                        kk = 4 * q4 + mq
                        nc.tensor.matmul(
                            psA[:, EX * mq:EX * (mq + 1)],
                            bank_sb[:, 128 * kk:128 * (kk + 1)],
                            balT_sb[:],
                            start=True, stop=True,
                        )
                    nc.vector.tensor_copy(
                        resT_sb[:, 64 * (4 * gl + q4):64 * (4 * gl + q4) + 64], psA[:]
                    )

            # ---- Phase B1: bandpass for all examples, park impulse in DRAM ----
            for b in range(EX):
                x2_sb = work.tile([128, ICH + 4], BF16, tag="x2")
                nc.sync.dma_start(out=x2_sb[:], in_=x2in.ap()[b])
                gs_sb = work.tile([128, WBP], BF16, tag="gs")
                nc.sync.dma_start(out=gs_sb[:], in_=gsin.ap()[b])
                acc2 = ps2.tile([ICH, 128], F32, tag="acc2")
                for qi in range(QBP):
                    nc.tensor.matmul(
                        acc2[:],
                        x2_sb[:, qi:qi + ICH],
                        gs_sb[:, 128 * qi:128 * (qi + 1)],
                        start=(qi == 0),
                        stop=(qi == QBP - 1),
                    )
                # reversed copy: imp_sb[n2, dt] = acc2[n2, 127 - dt]
                imp_sb = work.tile([ICH, 128], BF16, tag="imp")
                rev = acc2[:].copy()
                dims = [tuple(x) for x in rev.ap]
                rev.ap = V([dims[0], [-1, 128]])
                rev.offset = rev.offset + 127
                nc.vector.tensor_copy(imp_sb[:], rev)
                nc.gpsimd.dma_start(
                    out=h_buf[b][128:128 + IMP].rearrange("(n j) -> n j", j=128),
                    in_=imp_sb[:],
                )
                nc.gpsimd.dma_start(
                    out=impo.ap()[b].rearrange("(n j) -> n j", j=128), in_=imp_sb[:]
                )

            # ---- Phase B2: big conv per example ----
            X_tiles = []
            for k in range(3):
                Xt = constp.tile([128, NCH + 32], BF16, tag=f"Xt{k}")
                nc.vector.memset(Xt[:], 0.0)
                X_tiles.append(Xt)
            for b in range(EX):
                # h_shift[j, c] = h_buf[b, 1 + c + j]  (overlapping read)
                hs = hsp.tile([128, WBIG], BF16, tag="hs")
                hsrc = h_buf[:].copy()
                hsrc.ap = V([[1, 128], [1, WBIG]])
                hsrc.offset = b * HBUF + 1
                nc.sync.dma_start(out=hs[:], in_=hsrc)

                # X[j, 16 + m] = res_sig chunks = resT_sb[:, 16*m + b] * env
                env_sb = work.tile([128, NCH], BF16, tag="env")
                nc.scalar.dma_start(out=env_sb[:], in_=envv.ap()[b])
                X = X_tiles[b % 3]
                rsrc = resT_sb[:].copy()
                dims = [tuple(x) for x in rsrc.ap]
                rsrc.ap = V([dims[0], [EX, NCH]])
                rsrc.offset = rsrc.offset + b
                nc.vector.tensor_tensor(
                    X[:, 16:16 + NCH], rsrc, env_sb[:], mybir.AluOpType.mult
                )

                # big conv: out_rev[dt', n] accumulated over 33 Toeplitz blocks
                acc = ps1.tile([128, NCH], F32, tag="acc")
                for qi in range(QBIG):
                    nc.tensor.matmul(
                        acc[:],
                        hs[:, 128 * qi:128 * (qi + 1)],
                        X[:, qi:qi + NCH],
                        start=(qi == 0),
                        stop=(qi == QBIG - 1),
                    )
                out_sb = outs.tile([128, NCH], F32, tag="out")
                nc.vector.tensor_copy(out_sb[:], acc[:])
                nc.gpsimd.dma_start(out=outp.ap()[b], in_=out_sb[:])

    return nc


_PROGRAM = None


def _get_program():
    global _PROGRAM
    if _PROGRAM is None:
        _PROGRAM = _build_program()
    return _PROGRAM


def _hamming(n):
    return (0.54 - 0.46 * np.cos(2.0 * np.pi * np.arange(n) / n)).astype(np.float32)


def _host_prep(balance, decay, impulse_choice, filter_choice, amp, bank, filters,
               impulses):
    bf16 = ml_dtypes.bfloat16
    ham = _hamming(WIN)

    # noise must match jax.random.uniform(key(42), ...) bit-for-bit
    import jax
    import jax.numpy as jnp
    cpu = jax.devices("cpu")[0]
    with jax.default_device(cpu):
        noise = np.asarray(
            jax.random.uniform(
                jax.random.key(42), (B, IMP), minval=-1.0, maxval=1.0,
                dtype=jnp.float32,
            )
        )

    frames = (impulse_choice @ (impulses ** 2)).astype(np.float32)
    nfr = frames.shape[-1]
    pos = np.clip((np.arange(IMP) + 0.5) * (nfr / IMP) - 0.5, 0.0, nfr - 1.0)
    i0 = np.floor(pos).astype(np.int32)
    i1 = np.minimum(i0 + 1, nfr - 1)
    w = (pos - i0).astype(np.float32)
    env_imp = frames[:, i0] * (1.0 - w) + frames[:, i1] * w
    imp_sig = (env_imp * noise * amp[:, None]).astype(np.float32)

    # resonance envelope: env[b, t] = d^(w+1) ham[r] + [w>=1] d^w ham[r+256]
    d = (np.clip(decay.astype(np.float64), 0.0, 1.0) + 1e-8)[:, 0]
    nwin = N // HOP
    P = d[:, None] ** np.arange(0, nwin + 2)[None, :]
    tw = np.arange(N) // HOP
    r = np.arange(N) % HOP
    env = (P[:, tw + 1] * ham[r]).astype(np.float32)
    env[:, HOP:] += (P[:, tw[HOP:]] * ham[r[HOP:] + HOP]).astype(np.float32)
    env_jm = env.reshape(B, NCH, 128).transpose(0, 2, 1)

    filt = ((filter_choice @ filters) * _hamming(FILT)).astype(np.float32)
    g_buf = np.zeros((B, GBUF), np.float32)
    g_buf[:, 128:128 + FILT] = filt
    gs = sliding_window_view(g_buf, WBP, axis=1)[:, 1:129, :]

    x2 = np.zeros((B, 128, ICH + 4), np.float32)
    x2[:, :, 2:2 + ICH] = imp_sig.reshape(B, ICH, 128).transpose(0, 2, 1)

    in_maps = []
    for c in range(NCORES):
        s = slice(c * EX, (c + 1) * EX)
        in_maps.append({
            "balT": np.ascontiguousarray(balance[s].T).astype(bf16),
            "bank": bank.astype(bf16),
            "envv": np.ascontiguousarray(env_jm[s]).astype(bf16),
            "x2in": np.ascontiguousarray(x2[s]).astype(bf16),
            "gsin": np.ascontiguousarray(gs[s]).astype(bf16),
        })
    return in_maps


def _gather(results):
    out = np.empty((B, 1, N), np.float32)
    for c in range(NCORES):
        rev = results[c]["outp"]                      # [EX, 128(dt'), 256(n)]
        sig = rev.transpose(0, 2, 1)[:, :, ::-1].reshape(EX, N)
        sig[:, :IMP] += results[c]["impo"].astype(np.float32)
        out[c * EX:(c + 1) * EX, 0, :] = sig
    return out


def run(trace=False, **inputs):
    in_maps = _host_prep(**{k: np.asarray(v) for k, v in inputs.items()})
    nc = _get_program()
    res = bass_utils.run_bass_kernel_spmd(
        nc, in_maps, core_ids=list(range(NCORES)), trace=trace
    )
    return _gather(res.results), res


def kernel(**inputs):
    out, _ = run(trace=False, **inputs)
    return out


# revision 10
# speedup vs baseline: 1.0571x; 1.0571x over previous
"""Trainium2 Bass kernel for nn_Event: per-example resonance synthesis.

Math (see derivation in comments below):
  res   = balance @ bank                                   (B=128, N=32768)
  res_sig = res * env          -- the window/decay overlap-add collapses to an
                                  elementwise envelope since decay is constant
                                  per example and hamming windows overlap 2:1
  impulse = bandpass(imp_sig, filt)   512-tap grouped correlation
  out = (res_sig (*) impulse) + pad(impulse), all scaled by amp
        where (*) is a 4096-tap grouped correlation with pad 2048.

Device strategy (8 cores, 16 examples each, pure data parallel):
  - correlations are computed on the TensorEngine as 33 (resp. 5)
    accumulating 128x128 Toeplitz-block matmuls per example.  The Toeplitz
    weight tiles are slices of a "shifted replication" matrix
    h_shift[j, c] = h_pad[c + j] read from DRAM with an overlapping
    (element-stride) access pattern, one DMA per example.
  - outputs are produced in a within-chunk-reversed layout [dt', n] with
    t = 128*n + 127 - dt' (this keeps every DMA/AP stride positive); the
    host undoes the permutation when gathering.
  - amp is folded into imp_sig on the host, so conv and impulse come out
    pre-scaled; the host adds the impulse head into the final signal.
"""

import numpy as np
import ml_dtypes
from numpy.lib.stride_tricks import sliding_window_view

import concourse.bass as bass
import concourse.tile as tile
import concourse.mybir as mybir
import bass_rust
from concourse import bass_utils
from concourse.vector_clock import ScopedClock

V = bass_rust.VecI64Pair
BF16 = mybir.dt.bfloat16
F32 = mybir.dt.float32

B = 128
N = 32768
NF = 128
IMP = 4096
FILT = 512
WIN = 512
HOP = 256
NCORES = 8
EX = B // NCORES          # 16 examples per core
NCH = N // 128            # 256 signal chunks
ICH = IMP // 128          # 32 impulse chunks
QBIG = 33                 # Toeplitz blocks for the 4096-tap conv
QBP = 5                   # Toeplitz blocks for the 512-tap bandpass
WBIG = QBIG * 128         # 4224
WBP = QBP * 128           # 640
HBUF = 128 + IMP + 384    # padded h buffer per example (4608)
GBUF = 128 + FILT + 256   # padded g buffer per example (896)


class PatchedTileContext(tile.TileContext):
    """TileContext adapted to a walrus build that rejects more than one sync
    wait per instruction: extra waits are peeled onto standalone single-wait
    nops committed just before their instruction; same for the tail drain."""

    def _commit_instruction(self, inst, lazy_reg_writes: bool = True):
        si = getattr(inst, "sync_info", None)
        if (
            si is not None
            and si.on_wait
            and len(si.on_wait) > 1
            and inst.engine != mybir.EngineType.Unassigned
        ):
            waits = list(si.on_wait)
            inst.sync_info = mybir.SyncInfo(
                on_wait=waits[-1:], on_update=list(si.on_update or [])
            )
            for w in waits[:-1]:
                n = mybir.InstNoOp(
                    name=self.nc.get_next_instruction_name(),
                    sync_info=mybir.SyncInfo(on_wait=[w], on_update=[]),
                    bass_nofuse=True,
                    engine=inst.engine,
                    text_hint="split_wait",
                )
                super()._commit_instruction(n, lazy_reg_writes=False)
        return super()._commit_instruction(inst, lazy_reg_writes)

    def _drain_and_barrier(self, tick_clock, wait_clock):
        nc = self.nc
        drain_inst = nc.sync.drain()
        wait_clock.add_sem_waits(
            drain_inst.ins, ScopedClock({None: tick_clock.global_clock})
        )
        si = drain_inst.ins.sync_info
        waits = list(si.on_wait) if si is not None and si.on_wait else []
        if len(waits) > 1:
            si.on_wait = []
            for w in waits:
                n = nc.sync.nop(nofuse=True, hint="split_drain_wait")
                n.ins.sync_info = mybir.SyncInfo(on_wait=[w], on_update=[])
        nc.all_engine_barrier()
        assert self.sems is not None
        popped = nc._tile_sem_poison_stack.pop()
        assert popped is self._sem_poison
        nc.clear_and_free_semaphores(list(self.sems.allocated().values()))
        nc.all_engine_barrier()


def _build_program():
    nc = bass.Bass("TRN2", target_bir_lowering=False, debug=False)

    balT = nc.dram_tensor("balT", [NF, EX], BF16, kind="ExternalInput")
    bank = nc.dram_tensor("bank", [NF, N], BF16, kind="ExternalInput")
    envv = nc.dram_tensor("envv", [EX, 128, NCH], BF16, kind="ExternalInput")
    x2in = nc.dram_tensor("x2in", [EX, 128, ICH + 4], BF16, kind="ExternalInput")
    gsin = nc.dram_tensor("gsin", [EX, 128, WBP], BF16, kind="ExternalInput")
    outp = nc.dram_tensor("outp", [EX, 128, NCH], F32, kind="ExternalOutput")
    impo = nc.dram_tensor("impo", [EX, IMP], BF16, kind="ExternalOutput")

    with PatchedTileContext(nc) as tc:
        with (
            tc.tile_pool(name="const", bufs=1) as constp,
            tc.tile_pool(name="dram", bufs=1, space="DRAM") as dramp,
            tc.tile_pool(name="bankp", bufs=6) as bankp,
            tc.tile_pool(name="work", bufs=3) as work,
            tc.tile_pool(name="hsp", bufs=4) as hsp,
            tc.tile_pool(name="outs", bufs=3) as outs,
            tc.tile_pool(name="psr", bufs=3, space="PSUM") as psr,
            tc.tile_pool(name="ps1", bufs=2, space="PSUM") as ps1,
            tc.tile_pool(name="ps2", bufs=2, space="PSUM") as ps2,
        ):
            h_buf = dramp.tile([EX, HBUF], BF16)

            balT_sb = constp.tile([NF, EX], BF16)
            nc.sync.dma_start(out=balT_sb[:], in_=balT.ap())

            zero_sb = constp.tile([EX, 384], BF16)
            nc.vector.memset(zero_sb[:], 0.0)
            nc.gpsimd.dma_start(out=h_buf[:, 0:128], in_=zero_sb[:, 0:128])
            nc.gpsimd.dma_start(out=h_buf[:, 128 + IMP:HBUF], in_=zero_sb[:])

            # ---- Phase A: resT[t, b] = (balance @ bank).T, kept on-chip ----
            # resT_sb column 16*m + b holds res[b, 128*m + j] on partition j.
            resT_sb = constp.tile([128, EX * NCH], BF16)
            for gl in range(16):
                bank_sb = bankp.tile([NF, 2048], BF16, tag="bank_sb")
                ldeng = nc.scalar if gl % 2 == 0 else nc.sync
                ldeng.dma_start(out=bank_sb[:], in_=bank.ap()[:, 2048 * gl:2048 * (gl + 1)])
                for q4 in range(4):
                    psA = psr.tile([128, 4 * EX], F32, tag="psA")
                    for mq in range(4):
                        kk = 4 * q4 + mq
                        nc.tensor.matmul(
                            psA[:, EX * mq:EX * (mq + 1)],
                            bank_sb[:, 128 * kk:128 * (kk + 1)],
                            balT_sb[:],
                            start=True, stop=True,
                        )
                    nc.vector.tensor_copy(
                        resT_sb[:, 64 * (4 * gl + q4):64 * (4 * gl + q4) + 64], psA[:]
                    )

            # ---- Phase B1: bandpass for all examples, park impulse in DRAM ----
            for b in range(EX):
                x2_sb = work.tile([128, ICH + 4], BF16, tag="x2")
                nc.sync.dma_start(out=x2_sb[:], in_=x2in.ap()[b])
                gs_sb = work.tile([128, WBP], BF16, tag="gs")
                nc.sync.dma_start(out=gs_sb[:], in_=gsin.ap()[b])
                acc2 = ps2.tile([ICH, 128], F32, tag="acc2")
                for qi in range(QBP):
                    nc.tensor.matmul(
                        acc2[:],
                        x2_sb[:, qi:qi + ICH],
                        gs_sb[:, 128 * qi:128 * (qi + 1)],
                        start=(qi == 0),
                        stop=(qi == QBP - 1),
                    )
                # reversed copy: imp_sb[n2, dt] = acc2[n2, 127 - dt]
                imp_sb = work.tile([ICH, 128], BF16, tag="imp")
                rev = acc2[:].copy()
                dims = [tuple(x) for x in rev.ap]
                rev.ap = V([dims[0], [-1, 128]])
                rev.offset = rev.offset + 127
                nc.vector.tensor_copy(imp_sb[:], rev)
                nc.gpsimd.dma_start(
                    out=h_buf[b][128:128 + IMP].rearrange("(n j) -> n j", j=128),
                    in_=imp_sb[:],
                )
                nc.gpsimd.dma_start(
                    out=impo.ap()[b].rearrange("(n j) -> n j", j=128), in_=imp_sb[:]
                )

            # ---- Phase B2: big conv per example ----
            X_tiles = []
            for k in range(3):
                Xt = constp.tile([128, NCH + 32], BF16, tag=f"Xt{k}")
                nc.vector.memset(Xt[:], 0.0)
                X_tiles.append(Xt)
            for b in range(EX):
                # h_shift[j, c] = h_buf[b, 1 + c + j]  (overlapping read)
                hs = hsp.tile([128, WBIG], BF16, tag="hs")
                hsrc = h_buf[:].copy()
                hsrc.ap = V([[1, 128], [1, WBIG]])
                hsrc.offset = b * HBUF + 1
                nc.sync.dma_start(out=hs[:], in_=hsrc)

                # X[j, 16 + m] = res_sig chunks = resT_sb[:, 16*m + b] * env
                env_sb = work.tile([128, NCH], BF16, tag="env")
                nc.scalar.dma_start(out=env_sb[:], in_=envv.ap()[b])
                X = X_tiles[b % 3]
                rsrc = resT_sb[:].copy()
                dims = [tuple(x) for x in rsrc.ap]
                rsrc.ap = V([dims[0], [EX, NCH]])
                rsrc.offset = rsrc.offset + b
                nc.vector.tensor_tensor(
                    X[:, 16:16 + NCH], rsrc, env_sb[:], mybir.AluOpType.mult
                )

                # big conv: out_rev[dt', n] accumulated over 33 Toeplitz blocks
                acc = ps1.tile([128, NCH], F32, tag="acc")
                for qi in range(QBIG):
                    nc.tensor.matmul(
                        acc[:],
                        hs[:, 128 * qi:128 * (qi + 1)],
                        X[:, qi:qi + NCH],
                        start=(qi == 0),
                        stop=(qi == QBIG - 1),
                    )
                out_sb = outs.tile([128, NCH], F32, tag="out")
                nc.vector.tensor_copy(out_sb[:], acc[:])
                nc.gpsimd.dma_start(out=outp.ap()[b], in_=out_sb[:])

    return nc


_PROGRAM = None


def _get_program():
    global _PROGRAM
    if _PROGRAM is None:
        _PROGRAM = _build_program()
    return _PROGRAM


def _hamming(n):
    return (0.54 - 0.46 * np.cos(2.0 * np.pi * np.arange(n) / n)).astype(np.float32)


def _host_prep(balance, decay, impulse_choice, filter_choice, amp, bank, filters,
               impulses):
    bf16 = ml_dtypes.bfloat16
    ham = _hamming(WIN)

    # noise must match jax.random.uniform(key(42), ...) bit-for-bit
    import jax
    import jax.numpy as jnp
    cpu = jax.devices("cpu")[0]
    with jax.default_device(cpu):
        noise = np.asarray(
            jax.random.uniform(
                jax.random.key(42), (B, IMP), minval=-1.0, maxval=1.0,
                dtype=jnp.float32,
            )
        )

    frames = (impulse_choice @ (impulses ** 2)).astype(np.float32)
    nfr = frames.shape[-1]
    pos = np.clip((np.arange(IMP) + 0.5) * (nfr / IMP) - 0.5, 0.0, nfr - 1.0)
    i0 = np.floor(pos).astype(np.int32)
    i1 = np.minimum(i0 + 1, nfr - 1)
    w = (pos - i0).astype(np.float32)
    env_imp = frames[:, i0] * (1.0 - w) + frames[:, i1] * w
    imp_sig = (env_imp * noise * amp[:, None]).astype(np.float32)

    # resonance envelope: env[b, t] = d^(w+1) ham[r] + [w>=1] d^w ham[r+256]
    d = (np.clip(decay.astype(np.float64), 0.0, 1.0) + 1e-8)[:, 0]
    nwin = N // HOP
    P = d[:, None] ** np.arange(0, nwin + 2)[None, :]
    tw = np.arange(N) // HOP
    r = np.arange(N) % HOP
    env = (P[:, tw + 1] * ham[r]).astype(np.float32)
    env[:, HOP:] += (P[:, tw[HOP:]] * ham[r[HOP:] + HOP]).astype(np.float32)
    env_jm = env.reshape(B, NCH, 128).transpose(0, 2, 1)

    filt = ((filter_choice @ filters) * _hamming(FILT)).astype(np.float32)
    g_buf = np.zeros((B, GBUF), np.float32)
    g_buf[:, 128:128 + FILT] = filt
    gs = sliding_window_view(g_buf, WBP, axis=1)[:, 1:129, :]

    x2 = np.zeros((B, 128, ICH + 4), np.float32)
    x2[:, :, 2:2 + ICH] = imp_sig.reshape(B, ICH, 128).transpose(0, 2, 1)

    in_maps = []
    for c in range(NCORES):
        s = slice(c * EX, (c + 1) * EX)
        in_maps.append({
            "balT": np.ascontiguousarray(balance[s].T).astype(bf16),
            "bank": bank.astype(bf16),
            "envv": np.ascontiguousarray(env_jm[s]).astype(bf16),
            "x2in": np.ascontiguousarray(x2[s]).astype(bf16),
            "gsin": np.ascontiguousarray(gs[s]).astype(bf16),
        })
    return in_maps


def _gather(results):
    out = np.empty((B, 1, N), np.float32)
    for c in range(NCORES):
        rev = results[c]["outp"]                      # [EX, 128(dt'), 256(n)]
        sig = rev.transpose(0, 2, 1)[:, :, ::-1].reshape(EX, N)
        sig[:, :IMP] += results[c]["impo"].astype(np.float32)
        out[c * EX:(c + 1) * EX, 0, :] = sig
    return out


def run(trace=False, **inputs):
    in_maps = _host_prep(**{k: np.asarray(v) for k, v in inputs.items()})
    nc = _get_program()
    res = bass_utils.run_bass_kernel_spmd(
        nc, in_maps, core_ids=list(range(NCORES)), trace=trace
    )
    return _gather(res.results), res


def kernel(**inputs):
    out, _ = run(trace=False, **inputs)
    return out
